# revision 27
# baseline (speedup 1.0000x reference)
"""Trainium2 Bass kernel for nn_ConvLayer_13967233646751 (gnn_message_passing).

Reference computation (per batch b, point p, neighbor s):
  - build local frame R from normal + azimuth (mean of rel coords over s=1..31)
  - x = [R@rel, feats, R@other_normal, R@azi_u - R@other_dir]   (73 ch)
  - h = relu(W2 @ relu(W1 @ x + b1) + b2); pooled = max over s
  - out = concat([azi_u, pooled])  -> (B, 131, P)

Sharding: data-parallel over batch B=16 across 8 cores (2 batches/core).

V2 design (per core):
  xt per batch: [73, 32768] fp16, column = s*1024 + q*128 + pt  (s-plane major)
    rows 0:64  <- host-staged feats fp16, 4 DMAs of 1MB (16KB contiguous runs)
    rows 64:73 <- aligned geo via ONE XBAR dma_start_transpose from alq
                  (alq [128pt, 9c, 32s, 8q] fp16 -> rows (c,s,q) -> [9, 256, 128])
  rotation: fp16 DVE tensor_tensor with q innermost (packed) so 2x/4x modes apply;
    geo staged with dir NEGATED so dir channels become svec + R@(-dir).
  phase B: 32 s-plane chunks of 1024 cols; per plane mm1(2x512)->ACT relu+b1->
    mm2(2x512); pooling = TT-max of PSUM plane pairs -> fp16 slots (4) ->
    3-op tree -> ACT relu+b2 finalize.  Pooling is max over s = max over planes.
"""

import numpy as np
from contextlib import ExitStack

import concourse.bass as bass
import concourse.tile as tile
from concourse import bacc
from concourse import mybir
from concourse.bass_utils import run_bass_kernel_spmd

F32 = mybir.dt.float32
F16 = mybir.dt.float16
AX = mybir.AxisListType
OP = mybir.AluOpType
AF = mybir.ActivationFunctionType

EPS = 1e-8
B, C, P, S = 16, 76, 1024, 32
NCORES = 8
BL = B // NCORES          # batches per core
NQ = P // 128             # 8 q-groups per batch
NPLANES = S               # 32 s-planes per batch, 1024 cols each

# geo channel c = 3f+i -> w1 column (f: 0=nrm, 1=rel, 2=dir)
GEO_W1_COLS = [67, 68, 69, 0, 1, 2, 70, 71, 72]

# TT may read only ONE input from PSUM (NCC_IBVF027) and GPSIMD cannot
# touch PSUM at all, so pooling is a per-plane running max on DVE:
# slot(fp16 SBUF) = max(slot, h2_plane).  For CAST_PLANE planes the PSUM
# read goes through ACT (Copy -> fp16 SBUF) and DVE accumulates at 2x.
CAST_PLANE = lambda k: (k % 3) == 1
# planes whose h1 evacuation runs on vector (tensor_scalar) instead of scalar
DVE_H1_PLANES = ()


def _unit_ops(nc, v, u, sq, ss, nrm, inv):
    """u = v / (||v||+eps) along xyz; v,u,sq: [128,3,NQ]; ss,nrm,inv: [128,NQ]."""
    nc.vector.tensor_tensor(out=sq[:], in0=v[:], in1=v[:], op=OP.mult)
    nc.vector.reduce_sum(out=ss[:], in_=sq[:].transpose([0, 2, 1]), axis=AX.X)
    nc.scalar.sqrt(nrm[:], ss[:])
    nc.vector.tensor_scalar_add(out=nrm[:], in0=nrm[:], scalar1=EPS)
    nc.vector.reciprocal(inv[:], nrm[:])
    inv_b = inv[:].unsqueeze(1).broadcast_to([128, 3, NQ])
    nc.vector.tensor_tensor(out=u[:], in0=v[:], in1=inv_b, op=OP.mult)


def build_program():
    nc = bacc.Bacc()

    feats_d = nc.dram_tensor("featsT", [BL, 64, S, NQ, 128], F16, kind="ExternalInput")
    geo_d = nc.dram_tensor("geoT16", [BL, 128, 3, 3, S, NQ], F16, kind="ExternalInput")
    norm_d = nc.dram_tensor("normp", [128, BL, 3, NQ], F32, kind="ExternalInput")
    w1c_d = nc.dram_tensor("w1c", [73, 128], F16, kind="ExternalInput")
    w2T_d = nc.dram_tensor("w2T", [128, 128], F16, kind="ExternalInput")
    b1_d = nc.dram_tensor("b1c", [128, 1], F32, kind="ExternalInput")
    b2_d = nc.dram_tensor("b2c", [128, 1], F32, kind="ExternalInput")
    outp_d = nc.dram_tensor("outp", [BL, 128, P], F32, kind="ExternalOutput")
    outa_d = nc.dram_tensor("outa", [BL, 128, 3, NQ], F32, kind="ExternalOutput")

    with tile.TileContext(nc) as tc, ExitStack() as ctx:
        cpool = ctx.enter_context(tc.tile_pool(name="const", bufs=1))
        g16_pool = ctx.enter_context(tc.tile_pool(name="g16", bufs=2))
        rpool = ctx.enter_context(tc.tile_pool(name="rphase", bufs=2))
        alq_pool = ctx.enter_context(tc.tile_pool(name="alq", bufs=2))
        xt_pool = ctx.enter_context(tc.tile_pool(name="xt", bufs=2))
        h1_pool = ctx.enter_context(tc.tile_pool(name="h1", bufs=3))
        slot_pool = ctx.enter_context(tc.tile_pool(name="slots", bufs=2))
        po_pool = ctx.enter_context(tc.tile_pool(name="pooled", bufs=2))
        ps1_pool = ctx.enter_context(tc.tile_pool(name="ps1", bufs=2, space="PSUM"))
        ps2_pool = ctx.enter_context(tc.tile_pool(name="ps2", bufs=2, space="PSUM"))

        # ---- constants (scalar queue; sync stays clear for xbar/scatter) ----
        norm_pt = cpool.tile([128, BL, 3, NQ], F32)
        nc.scalar.dma_start(out=norm_pt[:], in_=norm_d[:])
        b1t = cpool.tile([128, 1], F32)
        nc.scalar.dma_start(out=b1t[:], in_=b1_d[:, :])
        b2t = cpool.tile([128, 1], F32)
        nc.scalar.dma_start(out=b2t[:], in_=b2_d[:, :])
        w1c = cpool.tile([73, 128], F16)
        nc.scalar.dma_start(out=w1c[:], in_=w1c_d[:, :])
        w2T = cpool.tile([128, 128], F16)
        nc.scalar.dma_start(out=w2T[:], in_=w2T_d[:, :])

        xts = {}
        for b in range(BL):
            # ---- feats loads into xt rows 0:64 (big; start them early) ----
            # ---- geo load FIRST (critical path: azi -> R -> rotation);
            # host-staged fp16 in rotation layout, dir negated, rel (f=1)
            # first so the azimuth reduction starts early ----
            geo16 = g16_pool.tile([128, 3, 3, S, NQ], F16, tag="geo16")
            for f in (1, 0, 2):
                nc.gpsimd.dma_start(out=geo16[:, f], in_=geo_d[b, :, f])

            xt = xt_pool.tile([73, NPLANES * 1024], F16, tag="xt", name=f"xt_{b}")
            xts[b] = xt
            for blk in range(4):
                eng = (nc.scalar, nc.gpsimd)[blk % 2]
                eng.dma_start(
                    out=xt[0:64, 8192 * blk:8192 * (blk + 1)],
                    in_=feats_d[b, :, 8 * blk:8 * (blk + 1)].rearrange("c s q p -> c (s q p)"),
                )

            # ---- R phase (fp32 on DVE) -> R9 [128, 3i, 3j, 8q] ----
            R9 = rpool.tile([128, 3, 3, NQ], F32, tag="R9")
            azi = rpool.tile([128, 3, NQ], F32, tag="azi")
            sq = rpool.tile([128, 3, NQ], F32, tag="sq")
            ss = rpool.tile([128, NQ], F32, tag="ss")
            nrm = rpool.tile([128, NQ], F32, tag="nrm")
            inv = rpool.tile([128, NQ], F32, tag="inv")
            a_u = rpool.tile([128, 3, NQ], F32, tag="a_u")
            dot = rpool.tile([128, NQ], F32, tag="dot")
            xraw = rpool.tile([128, 3, NQ], F32, tag="xraw")
            svec = rpool.tile([128, 3, NQ], F32, tag="svec")
            tmp3 = rpool.tile([128, 3, NQ], F32, tag="tmp3")
            x_u = R9[:, 0]
            yax = R9[:, 1]
            n_u = R9[:, 2]

            # unit(normal) first -- independent of the geo DMA
            _unit_ops(nc, norm_pt[:, b], n_u, sq, ss, nrm, inv)

            # azi = mean over s=1..31 of rel (f=1); geo16 is [pt, f, j, s, q]
            for x_ in range(3):
                nc.vector.reduce_sum(out=azi[:, x_, :],
                                     in_=geo16[:, 1, x_, 1:S, :].transpose([0, 2, 1]),
                                     axis=AX.X)
            nc.vector.tensor_scalar_mul(out=azi[:], in0=azi[:], scalar1=1.0 / 31.0)

            _unit_ops(nc, azi, a_u, sq, ss, nrm, inv)

            # dot = sum_xyz a_u*n_u
            nc.vector.tensor_tensor(out=tmp3[:], in0=a_u[:], in1=n_u[:], op=OP.mult)
            nc.vector.reduce_sum(out=dot[:], in_=tmp3[:].transpose([0, 2, 1]), axis=AX.X)

            # xraw = a_u - dot*n_u
            dot_b = dot[:].unsqueeze(1).broadcast_to([128, 3, NQ])
            nc.vector.tensor_tensor(out=xraw[:], in0=dot_b, in1=n_u[:], op=OP.mult)
            nc.vector.tensor_tensor(out=xraw[:], in0=a_u[:], in1=xraw[:], op=OP.subtract)
            _unit_ops(nc, xraw, x_u, sq, ss, nrm, inv)

            # yax = cross(n_u, x_u)
            for x_ in range(3):
                i1, i2 = (x_ + 1) % 3, (x_ + 2) % 3
                nc.vector.tensor_tensor(out=yax[:, x_, :], in0=n_u[:, i1, :], in1=x_u[:, i2, :], op=OP.mult)
                nc.vector.tensor_tensor(out=tmp3[:, x_, :], in0=n_u[:, i2, :], in1=x_u[:, i1, :], op=OP.mult)
            nc.vector.tensor_tensor(out=yax[:], in0=yax[:], in1=tmp3[:], op=OP.subtract)

            # svec_i = R_i . a_u  (svec_2 = dot)
            nc.vector.tensor_tensor(out=tmp3[:], in0=x_u[:], in1=a_u[:], op=OP.mult)
            nc.vector.reduce_sum(out=svec[:, 0, :], in_=tmp3[:].transpose([0, 2, 1]), axis=AX.X)
            nc.vector.tensor_tensor(out=tmp3[:], in0=yax[:], in1=a_u[:], op=OP.mult)
            nc.vector.reduce_sum(out=svec[:, 1, :], in_=tmp3[:].transpose([0, 2, 1]), axis=AX.X)
            nc.vector.tensor_copy(out=svec[:, 2, :], in_=dot[:])

            # azi_u output (host assembles rows 0:3)
            nc.sync.dma_start(out=outa_d[b], in_=a_u[:])

            # fp16 casts for the rotation
            R9h = rpool.tile([128, 3, 3, NQ], F16, tag="R9h")
            nc.vector.tensor_copy(out=R9h[:], in_=R9[:])
            svec16 = rpool.tile([128, 3, NQ], F16, tag="svec16")
            nc.vector.tensor_copy(out=svec16[:], in_=svec[:])

            # ---- rotation + transpose, split into s-halves so the first 16
            # planes' GEMMs can start while the second half still rotates.
            # alq[pt, hi, c=3f+i, slo, q]; per half: XBAR rows f = c*128+slo*8+q
            # land at partition (slo*8+q), slot c -> staging [128, 9, 128];
            # 9 wide SBUF->SBUF DMAs move channel rows into xt[64:73]. ----
            alq = alq_pool.tile([128, 2, 9, S // 2, NQ], F16, tag="alq")
            for hi in range(2):
                rtmp = rpool.tile([128, 3, S // 2, NQ], F16, tag=f"rtmp{hi}")
                # lo half on DVE (fast, gates the first 16 planes); hi half on
                # the otherwise-idle gpsimd (slower but off the critical path)
                reng = nc.vector if hi == 0 else nc.gpsimd
                slo = slice(16 * hi, 16 * hi + 16)
                for i in range(3):
                    out3 = alq[:, hi, i:9:3, :, :]            # c = i, 3+i, 6+i
                    for j in range(3):
                        rb = R9h[:, i, j, :].unsqueeze(1).unsqueeze(2).broadcast_to(
                            [128, 3, S // 2, NQ])
                        src = geo16[:, :, j, slo, :]          # [pt, f, slo, q]
                        if j == 0:
                            reng.tensor_tensor(out=out3, in0=src, in1=rb, op=OP.mult)
                        else:
                            reng.tensor_tensor(out=rtmp[:], in0=src, in1=rb, op=OP.mult)
                            reng.tensor_tensor(out=out3, in0=out3, in1=rtmp[:], op=OP.add)
                # dir channels (geo staged as -dir): c=6..8 += svec (gpsimd)
                sv_b = svec16[:].unsqueeze(2).broadcast_to([128, 3, S // 2, NQ])
                nc.gpsimd.tensor_tensor(out=alq[:, hi, 6:9, :, :],
                                        in0=alq[:, hi, 6:9, :, :], in1=sv_b, op=OP.add)
                tstg = alq_pool.tile([128, 9, 128], F16, tag=f"tstg{hi}")
                nc.sync.dma_start_transpose(
                    out=tstg[:], in_=alq[:, hi].rearrange("p c s q -> p (c s q)"))
                for c in range(9):
                    nc.sync.dma_start(
                        out=xt[64 + c:65 + c, 16384 * hi:16384 * (hi + 1)],
                        in_=tstg[:, c, :],
                    )

        # ---- phase B: GEMMs + pooling, per batch, 32 s-planes ----
        for b in range(BL):
            xt = xts[b]
            slots = slot_pool.tile([128, 4, 1024], F16, tag="slots")
            slot_init = [False] * 4
            for k in range(NPLANES):
                h1ps = ps1_pool.tile([128, 1024], F32, tag="h1ps")
                nc.tensor.matmul(out=h1ps[:, 0:512], lhsT=w1c[:],
                                 rhs=xt[:, 1024 * k:1024 * k + 512],
                                 start=True, stop=True)
                nc.tensor.matmul(out=h1ps[:, 512:1024], lhsT=w1c[:],
                                 rhs=xt[:, 1024 * k + 512:1024 * k + 1024],
                                 start=True, stop=True)
                h1sb = h1_pool.tile([128, 1024], F16, tag="h1sb")
                if k in DVE_H1_PLANES:
                    nc.vector.tensor_scalar(out=h1sb[:], in0=h1ps[:],
                                            scalar1=b1t[:, 0:1], scalar2=0.0,
                                            op0=OP.add, op1=OP.max)
                else:
                    nc.scalar.activation(h1sb[:], h1ps[:], AF.Relu, bias=b1t[:, 0:1])
                h2 = ps2_pool.tile([128, 1024], F32, tag="h2ps")
                nc.tensor.matmul(out=h2[:, 0:512], lhsT=w2T[:],
                                 rhs=h1sb[:, 0:512], start=True, stop=True)
                nc.tensor.matmul(out=h2[:, 512:1024], lhsT=w2T[:],
                                 rhs=h1sb[:, 512:1024], start=True, stop=True)
                # pooling: running max into an fp16 slot (PSUM read via ACT
                # cast for CAST_PLANEs, else directly on DVE)
                slot = slots[:, k % 4, :]
                if CAST_PLANE(k):
                    h2sb = h1_pool.tile([128, 1024], F16, tag="h2sb")
                    nc.scalar.activation(h2sb[:], h2[:], AF.Copy)
                    if not slot_init[k % 4]:
                        nc.vector.tensor_copy(out=slot, in_=h2sb[:])
                    else:
                        nc.vector.tensor_tensor(out=slot, in0=slot, in1=h2sb[:], op=OP.max)
                else:
                    if not slot_init[k % 4]:
                        nc.vector.tensor_scalar(out=slot, in0=h2[:], scalar1=0.0,
                                                scalar2=None, op0=OP.add)
                    else:
                        nc.vector.tensor_tensor(out=slot, in0=slot, in1=h2[:], op=OP.max)
                slot_init[k % 4] = True
            # tree: 4 slots -> 1 (fp16 2x on DVE), then relu(max + b2) -> fp32
            nc.vector.tensor_tensor(out=slots[:, 0:2, :], in0=slots[:, 0:2, :],
                                    in1=slots[:, 2:4, :], op=OP.max)
            nc.vector.tensor_tensor(out=slots[:, 0, :], in0=slots[:, 0, :],
                                    in1=slots[:, 1, :], op=OP.max)
            pooled_o = po_pool.tile([128, P], F32, tag="pooled_o")
            nc.scalar.activation(pooled_o[:], slots[:, 0, :], AF.Relu, bias=b2t[:, 0:1])
            nc.gpsimd.dma_start(out=outp_d[b], in_=pooled_o[:])

    nc.finalize()
    return nc


_CACHE = {}


def _get_program():
    if "nc" not in _CACHE:
        _CACHE["nc"] = build_program()
    return _CACHE["nc"]


def make_in_maps(input, normal, w1, b1, w2, b2):
    input = np.asarray(input, dtype=np.float32)
    normal = np.asarray(normal, dtype=np.float32)
    w1 = np.asarray(w1, dtype=np.float32)
    b1 = np.asarray(b1, dtype=np.float32)
    w2 = np.asarray(w2, dtype=np.float32)
    b2 = np.asarray(b2, dtype=np.float32)

    w1fT = w1[:, 3:67].T.astype(np.float16)                  # (64,128)
    w1gT = w1[:, GEO_W1_COLS].T.astype(np.float16)           # (9,128)
    w1c = np.ascontiguousarray(np.concatenate([w1fT, w1gT], axis=0))  # (73,128)
    w2T = np.ascontiguousarray(w2.T.astype(np.float16))      # (128,128)
    b1c = np.ascontiguousarray(b1.reshape(128, 1))
    b2c = np.ascontiguousarray(b2.reshape(128, 1))

    in_maps = []
    for core in range(NCORES):
        b0 = core * BL
        inp = input[b0:b0 + BL]
        # featsT[b, c, s, q, pt] = feats[b, c, q*128+pt, s]
        f = inp[:, 12:76].astype(np.float16)                 # (BL,64,1024,32)
        featsT = np.ascontiguousarray(
            f.reshape(BL, 64, NQ, 128, S).transpose(0, 1, 4, 2, 3))
        # geoT16[b, pt, f, j, s, q] fp16 with dir block negated
        # (dir channels become svec + R@(-dir))
        geo = inp[:, 3:12].copy()                            # (BL,9,1024,32)
        geo[:, 6:9] *= -1.0
        geo = np.ascontiguousarray(
            geo.reshape(BL, 3, 3, NQ, 128, S).transpose(0, 4, 1, 2, 5, 3)
            .astype(np.float16))                             # (BL,128,3,3,S,NQ)
        # normp[pt, b, xyz, q] = normal[b, q*128+pt, xyz]
        normp = np.ascontiguousarray(
            normal[b0:b0 + BL].reshape(BL, NQ, 128, 3).transpose(2, 0, 3, 1))
        in_maps.append({
            "featsT": featsT, "geoT16": geo, "normp": normp,
            "w1c": w1c, "w2T": w2T, "b1c": b1c, "b2c": b2c,
        })
    return in_maps


def assemble_output(results):
    outs = []
    for r in results:
        outp = r["outp"]                      # (BL,128,P)
        outa = r["outa"]                      # (BL,128,3,NQ)
        azi = outa.transpose(0, 2, 3, 1).reshape(BL, 3, P)
        outs.append(np.concatenate([azi, outp], axis=1))   # (BL,131,P)
    return np.concatenate(outs, axis=0)


def kernel(input, normal, w1, b1, w2, b2, _trace=False):
    nc = _get_program()
    in_maps = make_in_maps(input, normal, w1, b1, w2, b2)
    res = run_bass_kernel_spmd(nc, in_maps, core_ids=list(range(NCORES)), trace=_trace)
    out = assemble_output(res.results)
    if _trace:
        return out, res
    return out
